# revision 50
# baseline (speedup 1.0000x reference)
"""Grouped-experts SwiGLU kernel for 8 Trainium2 NeuronCores.

Problem: x[E,T,D], w1[E,D,H], w2[E,H,D], w3[E,D,H] with E=8, T=1024,
D=1024, H=2048.  out_e = (silu(x_e @ w1_e) * (x_e @ w3_e)) @ w2_e.

Sharding: expert-parallel, one expert per NeuronCore (E == n_cores == 8).
Each core runs an identical Bass program on its expert's slices; no
collectives are needed and the full output is just the stack of the
per-core outputs.

Per-core schedule (all matmuls in float32r => full PE rate at N>=256):
  - A short chain of throwaway transposes on a memset tile warms the PE
    HAM clock gate while the first DMAs land.
  - x is loaded in natural [T,D] 128-row tiles and transposed 128x128 on
    the TensorEngine (f32r transpose = 1.5 cyc/row) into resident xT
    (partition = D).  Transposes run 4-to-a-PSUM-tile with one strided
    evacuation copy each.  Loads are interleaved at half-tile/half-slab
    granularity (x halves / first weight slabs / x tail), the first four
    H-tiles' weights are held resident, and the first stage-A chunks are
    256 tokens wide, so the PE starts computing ~6 us into the DMA
    stream and never starves on the x tail.
  - Stage A: for each of the 16 H-tiles, stream w1/w3 slices, compute
    gT/upT = w^T @ x^T in PSUM accumulating over the 8 D-chunks,
    silu on ScalarE, multiply on VectorE -> resident hT (partition = H).
  - Stage B: for each 256-wide D-chunk, stream w2 in 4-H-tile quad DMAs,
    compute out = h @ w2 accumulating over the 16 H-chunks; PSUM
    evacuated via alternating ScalarE/VectorE copies and DMAed out in
    natural [T,D] layout on the second DMA ring.
  - SBUF ring sharing: the startup x tiles and the stage-B w2 quads
    share one 8-slot pool (disjoint lifetimes), as do the transpose
    PSUM tiles and the stage-B accumulators (4 banks).
"""

import sys

if "/opt/trn_rl_repo" not in sys.path:
    sys.path.insert(0, "/opt/trn_rl_repo")

import numpy as np

E, T, D, H = 8, 1024, 1024, 2048
P = 128
NT, ND, NH = T // P, D // P, H // P
TC = 512  # stage-A moving (token) chunk in steady state
DC = 256  # stage-B moving (dim) chunk
NTC, NDC = T // TC, D // DC
NQ = 4  # stage-B w2 loads batch 4 H-tiles per DMA
NHQ = NH // NQ


def build_program(reps: int = 1):
    """Build the per-core Bass program. reps>1 repeats the whole compute
    body (for wall-clock slope timing); the result is identical."""
    import concourse.bacc as bacc
    import concourse.mybir as mybir
    from concourse import tile, masks

    f32 = mybir.dt.float32
    f32r = mybir.dt.float32r
    SILU = mybir.ActivationFunctionType.Silu

    nc = bacc.Bacc("TRN2", target_bir_lowering=False, debug=False)
    x_d = nc.declare_dram_parameter("x", [T, D], f32, isOutput=False)
    w1_d = nc.declare_dram_parameter("w1", [D, H], f32, isOutput=False)
    w2_d = nc.declare_dram_parameter("w2", [H, D], f32, isOutput=False)
    w3_d = nc.declare_dram_parameter("w3", [D, H], f32, isOutput=False)
    out_d = nc.declare_dram_parameter("out", [T, D], f32, isOutput=True)

    # DRAM views with the partition dim innermost of the leading axis.
    w1_v = w1_d[:].rearrange("(dd p) hh -> p dd hh", p=P)  # [128, ND, H]
    w3_v = w3_d[:].rearrange("(dd p) hh -> p dd hh", p=P)
    w2_v = w2_d[:].rearrange("(hh p) dd -> p hh dd", p=P)  # [128, NH, D]

    with tile.TileContext(nc) as tc:
        with (
            tc.tile_pool(name="const", bufs=1) as const_pool,
            tc.tile_pool(name="xT", bufs=1) as xT_pool,
            tc.tile_pool(name="hT", bufs=1) as hT_pool,
            # one ring shared by the startup x tiles and the stage-B w2
            # quads (same 4 KB size, disjoint lifetimes): gives 8-deep x
            # streaming early and 4-quad w2 prefetch later for free
            tc.tile_pool(name="stream", bufs=8) as stream_pool,
            tc.tile_pool(name="wA", bufs=8) as wA_pool,
            tc.tile_pool(name="sg", bufs=3) as sg_pool,
            tc.tile_pool(name="ob", bufs=4) as ob_pool,
            # psX is shared by the transposes (stage A prep) and the
            # stage-B output accumulators -- disjoint lifetimes, so both
            # get 4-deep pipelining from the same 4 PSUM banks
            tc.tile_pool(name="psX", bufs=4, space="PSUM") as psX_pool,
            tc.tile_pool(name="psA", bufs=4, space="PSUM") as psA_pool,
        ):
            # warm-up source: a zero f32 tile via DVE memset (ready ~1 us
            # before the identity), so the PE clock-gate warm-up can start
            # as early as possible
            warm_src = const_pool.tile([P, P], f32, name="warmsrc", tag="warmsrc")
            nc.vector.memset(warm_src[:], 0.0)

            ident_f = const_pool.tile([P, P], f32, name="identf", tag="identf")
            masks.make_identity(nc, ident_f[:])
            ident_t = const_pool.tile([P, P], f32r, name="ident", tag="ident")
            nc.vector.tensor_copy(ident_t[:], ident_f[:])
            ident = ident_t[:]

            for rep in range(reps):
                # one [P, ND, T] tile so a 4-dd transpose quad can be
                # evacuated from PSUM with a single strided copy
                xTa = xT_pool.tile([P, ND, T], f32r, name="xTa", tag="xTa")
                hT = [
                    hT_pool.tile([P, T], f32r, name=f"hT{hh}", tag=f"hT{hh}")
                    for hh in range(NH)
                ]

                def load_x(tt, split=False):
                    xs = stream_pool.tile([P, D], f32r, name="xs", tag="stream")
                    if split:
                        # two half-row DMAs so the first transposes can
                        # start as soon as the first 256 KB land
                        nc.sync.dma_start(
                            out=xs[:, : D // 2],
                            in_=x_d[tt * P : (tt + 1) * P, : D // 2].bitcast(f32r),
                        )
                        nc.sync.dma_start(
                            out=xs[:, D // 2 :],
                            in_=x_d[tt * P : (tt + 1) * P, D // 2 :].bitcast(f32r),
                        )
                    else:
                        nc.sync.dma_start(
                            out=xs[:], in_=x_d[tt * P : (tt + 1) * P, :].bitcast(f32r)
                        )
                    return xs

                def do_tr_q(tt, xs, half):
                    # 4 transposes into one PSUM tile, one strided evac copy
                    pq = psX_pool.tile([P, 4, P], f32r, name="pq", tag="psX")
                    for k in range(4):
                        dd = half * 4 + k
                        nc.tensor.transpose(
                            pq[:, k, :], xs[:, dd * P : (dd + 1) * P], ident
                        )
                    dst = xTa[:, half * 4 : (half + 1) * 4, tt * P : (tt + 1) * P]
                    if (tt + half) % 2 == 0:
                        nc.vector.tensor_copy(dst, pq[:])
                    else:
                        nc.scalar.copy(dst, pq[:])

                def do_tr(tt, xs):
                    do_tr_q(tt, xs, 0)
                    do_tr_q(tt, xs, 1)

                def dma_w(hh, split=False):
                    hs = slice(hh * P, (hh + 1) * P)
                    nh = ND // 2
                    w1s = wA_pool.tile([P, ND, P], f32r, name="w1s", tag="w1s")
                    w3s = wA_pool.tile([P, ND, P], f32r, name="w3s", tag="w3s")
                    if split:
                        # half-slab DMAs: the dd<4 matmuls of the first
                        # stage-A groups start one half-slab earlier
                        nc.sync.dma_start(
                            out=w1s[:, :nh], in_=w1_v[:, :nh, hs].bitcast(f32r)
                        )
                        nc.sync.dma_start(
                            out=w1s[:, nh:], in_=w1_v[:, nh:, hs].bitcast(f32r)
                        )
                        nc.sync.dma_start(
                            out=w3s[:, :nh], in_=w3_v[:, :nh, hs].bitcast(f32r)
                        )
                        nc.sync.dma_start(
                            out=w3s[:, nh:], in_=w3_v[:, nh:, hs].bitcast(f32r)
                        )
                    else:
                        nc.sync.dma_start(out=w1s[:], in_=w1_v[:, :, hs].bitcast(f32r))
                        nc.sync.dma_start(out=w3s[:], in_=w3_v[:, :, hs].bitcast(f32r))
                    return w1s, w3s

                def stage_a(hh, ws, lo, width):
                    w1s, w3s = ws
                    tok = slice(lo, lo + width)
                    g_ps = psA_pool.tile([P, width], f32, name="g_ps", tag="psA")
                    u_ps = psA_pool.tile([P, width], f32, name="u_ps", tag="psA")
                    for dd in range(ND):
                        nc.tensor.matmul(
                            g_ps[:],
                            w1s[:, dd, :],
                            xTa[:, dd, tok],
                            start=(dd == 0),
                            stop=(dd == ND - 1),
                        )
                    for dd in range(ND):
                        nc.tensor.matmul(
                            u_ps[:],
                            w3s[:, dd, :],
                            xTa[:, dd, tok],
                            start=(dd == 0),
                            stop=(dd == ND - 1),
                        )
                    sg = sg_pool.tile([P, width], f32, name="sg", tag="sg")
                    nc.scalar.activation(sg[:], g_ps[:], SILU)
                    nc.vector.tensor_mul(hT[hh][:, tok], sg[:], u_ps[:])

                # ---- PE warm-up -------------------------------------------
                # ~20 throwaway transposes of the identity keep the PE busy
                # while the first x/w DMAs land, so the HAM clock gate is
                # fully open (2.4 GHz) when real matmuls start.
                if rep == 0:
                    warm = psX_pool.tile([P, P], f32, name="warm", tag="psX")
                    for _ in range(22):
                        nc.tensor.transpose(warm[:], warm_src[:], warm_src[:])

                # ---- interleaved startup ----------------------------------
                # DMA ring order: x0(2) x1 | w0 | x2 x3 | w1 | w2 | w3 |
                #                 x4..x7 | w4..w15 | stage-B w2 slabs
                # PE order: tr0 tr1 A(h0,T0) tr2 tr3 A(h0..h3 on T0)
                #           tr4..7 A(h0..h3 on T1) then steady state.
                xs_t = [None] * NT
                # custom order for the first 3 MB: the g dd<4 matmuls of
                # A(h0) need only x0a + x1a + w1-first-half
                xs_t[0] = stream_pool.tile([P, D], f32r, name="xs", tag="stream")
                xs_t[1] = stream_pool.tile([P, D], f32r, name="xs", tag="stream")
                w1s0 = wA_pool.tile([P, ND, P], f32r, name="w1s", tag="w1s")
                w3s0 = wA_pool.tile([P, ND, P], f32r, name="w3s", tag="w3s")
                Dh, Nh = D // 2, ND // 2

                def _xh(tt, half):
                    lo = half * Dh
                    nc.sync.dma_start(
                        out=xs_t[tt][:, lo : lo + Dh],
                        in_=x_d[tt * P : (tt + 1) * P, lo : lo + Dh].bitcast(f32r),
                    )

                def _wh(ws, wv, half):
                    lo = half * Nh
                    nc.sync.dma_start(
                        out=ws[:, lo : lo + Nh],
                        in_=wv[:, lo : lo + Nh, 0 * P : 1 * P].bitcast(f32r),
                    )

                _xh(0, 0)
                _xh(1, 0)
                _wh(w1s0, w1_v, 0)
                _xh(0, 1)
                _xh(1, 1)
                _wh(w1s0, w1_v, 1)
                _wh(w3s0, w3_v, 0)
                _wh(w3s0, w3_v, 1)
                w_r = [None] * 4
                w_r[0] = (w1s0, w3s0)
                do_tr(0, xs_t[0])
                do_tr(1, xs_t[1])
                stage_a(0, w_r[0], 0, 256)  # needs tr0,tr1 + w0 only
                # a-halves of x2,x3 first: the q0 transposes + the dd<4
                # matmuls of A(h0,256:512) depend only on those
                xs_t[2] = stream_pool.tile([P, D], f32r, name="xs", tag="stream")
                xs_t[3] = stream_pool.tile([P, D], f32r, name="xs", tag="stream")
                _xh(2, 0)
                _xh(3, 0)
                _xh(2, 1)
                _xh(3, 1)
                w_r[1] = dma_w(1, split=True)
                do_tr_q(2, xs_t[2], 0)
                do_tr_q(3, xs_t[3], 0)
                do_tr_q(2, xs_t[2], 1)
                do_tr_q(3, xs_t[3], 1)
                stage_a(0, w_r[0], 256, 256)
                stage_a(1, w_r[1], 0, 256)
                stage_a(1, w_r[1], 256, 256)
                w_r[2] = dma_w(2)
                w_r[3] = dma_w(3)
                xs_t[4] = load_x(4)
                xs_t[5] = load_x(5)
                xs_t[6] = load_x(6)
                xs_t[7] = load_x(7)
                stage_a(2, w_r[2], 0, TC)
                stage_a(3, w_r[3], 0, TC)
                do_tr(4, xs_t[4])
                do_tr(5, xs_t[5])
                # 256-wide T1 chunks interleave with the last transposes so
                # the PE never waits on the final quad-evacuation copies:
                # [512:768] needs only tt4,tt5; [768:1024] needs tt6,tt7,
                # and the h1 [512:768] chunk covers tr7's copy latency
                stage_a(0, w_r[0], 512, 256)
                do_tr(6, xs_t[6])
                do_tr(7, xs_t[7])
                stage_a(1, w_r[1], 512, 256)
                stage_a(0, w_r[0], 768, 256)
                stage_a(1, w_r[1], 768, 256)
                for hh in range(2, 4):
                    stage_a(hh, w_r[hh], TC, TC)

                # ---- Stage A steady state ---------------------------------
                for hh in range(4, NH):
                    ws = dma_w(hh)
                    for c in range(NTC):
                        stage_a(hh, ws, c * TC, TC)

                # ---- Stage B: out = h @ w2 --------------------------------
                # DC=256 keeps the resident w2 slice small; 4 H-tiles per
                # DMA keeps the DMA-queue entry count low.
                for dc in range(NDC):
                    dcs = slice(dc * DC, (dc + 1) * DC)
                    w2q = []
                    for q in range(NHQ):
                        w2t = stream_pool.tile(
                            [P, NQ, DC], f32r, name="w2t", tag="stream"
                        )
                        nc.sync.dma_start(
                            out=w2t[:],
                            in_=w2_v[:, q * NQ : (q + 1) * NQ, dcs].bitcast(f32r),
                        )
                        w2q.append(w2t)
                    for t in range(NT):
                        o_ps = psX_pool.tile([P, DC], f32, name="o_ps", tag="psX")
                        for hh in range(NH):
                            nc.tensor.matmul(
                                o_ps[:],
                                hT[hh][:, t * P : (t + 1) * P],
                                w2q[hh // NQ][:, hh % NQ, :],
                                start=(hh == 0),
                                stop=(hh == NH - 1),
                            )
                        ob = ob_pool.tile([P, DC], f32, name="ob", tag="ob")
                        if t % 2 == 0:
                            nc.vector.tensor_copy(ob[:], o_ps[:])
                        else:
                            nc.scalar.copy(ob[:], o_ps[:])
                        nc.scalar.dma_start(
                            out=out_d[t * P : (t + 1) * P, dcs], in_=ob[:]
                        )

    nc.compile()
    return nc


_program_cache = {}


def _get_program(reps: int = 1):
    if reps not in _program_cache:
        _program_cache[reps] = build_program(reps)
    return _program_cache[reps]


def kernel(x, w1, w2, w3):
    from concourse.bass_utils import run_bass_kernel_spmd

    x = np.ascontiguousarray(np.asarray(x, dtype=np.float32))
    w1 = np.ascontiguousarray(np.asarray(w1, dtype=np.float32))
    w2 = np.ascontiguousarray(np.asarray(w2, dtype=np.float32))
    w3 = np.ascontiguousarray(np.asarray(w3, dtype=np.float32))

    nc = _get_program()
    in_maps = [
        {"x": x[e], "w1": w1[e], "w2": w2[e], "w3": w3[e]} for e in range(E)
    ]
    res = run_bass_kernel_spmd(nc, in_maps, list(range(E)))
    out = np.stack([res.results[e]["out"] for e in range(E)], axis=0)
    return out.astype(np.float32)
